# revision 12
# baseline (speedup 1.0000x reference)
"""Trainium2 Bass kernel for nn_MemoryBlock (sliding-window attention +
neural long-term memory + gated FFN), distributed over 8 NeuronCores.

Sharding: data-parallel over the flattened token axis. Core c owns a
contiguous 1024-token block (batch c//4, seq block c%4) plus, on tail cores
3/7, the 10 persistent-memory tokens of their batch. Weights are replicated
(host pre-transposes them); activations stay feature-major ([feat, tok]) on
chip so the matmul chain needs no transposes; the token-axis grad
contraction uses cheap PE transposes of its two operands. The [M,M]
surprise-gradient is AllReduce-summed across the 8 cores; window attention
overlaps the collective. LayerNorm stats along the feature (partition) axis
use ones-vector matmuls; per-token stats are broadcast back with k=1
matmuls. float32r (bit-identical to fp32, full PE rate) throughout.

SBUF is managed as five tag banks in one pool; tags are reused across
phases (Tile's slot tracking serializes reuse correctly):
  X0-7 [P,1040]: xsT -> LN1 temps/gred -> gateT -> ffs[16:24]
  A0-7 [P,1040]: knT -> h1T -> attn keytiles -> memoT -> LN2 temps/comb_d
                 -> ffs[24:32]
  B0-7 [P,1040]: kpT -> h2T -> moT -> y2/x2T
  C0-8 [P,1024]: kptok/diff -> xstok -> wnew -> ffs[0:8] -> LN3 temps
  D0-8 [P,1024]: kn temps -> h2tok -> y1/x1T -> ffs[8:16] -> LN3 out
"""
import os
import sys

for _p in ("/opt/trn_rl_repo", "/root/.axon_site/_ro/trn_rl_repo"):
    if os.path.isdir(_p) and _p not in sys.path:
        sys.path.insert(0, _p)

import numpy as np

NCORES = 8
B, S, H, FF = 2, 4096, 1024, 4096
WIN, SLOTS = 16, 10
P = 128
TOKX = 1024            # x tokens per core
HALO = 16
EXTRA = 16             # persistent tokens, padded (10 real)
TOKV = TOKX + EXTRA    # 1040 kv tokens per core; also xs length (halo+block)
HB = H // P            # 8
FFB = FF // P          # 32
NKV = B * (S + SLOTS)  # 8212 real kv tokens (grad scaling)
NTT = (TOKV + P - 1) // P  # 9 token tiles
EPS_LN = 1e-5
EPS_NORM = 1e-12
NEG = -1e30
SCL = float(H) ** -0.5

CH_V = [(0, 512), (512, 512), (1024, 16)]   # column chunks for TOKV
CH_X = [(0, 512), (512, 512)]               # column chunks for TOKX
PSTAGS = ["mmA", "mmB", "mmC"]


def _build(native_silu: bool):
    import concourse.bass as bass
    import concourse.bacc as bacc
    import concourse.tile as tile
    from concourse import mybir

    F32R = mybir.dt.float32r
    fp32 = mybir.dt.float32
    AF = mybir.ActivationFunctionType
    A = mybir.AluOpType

    nc = bacc.Bacc("TRN2", target_bir_lowering=False, debug=False,
                   num_devices=NCORES)

    def din(name, shape, dt=fp32):
        return nc.dram_tensor(name, list(shape), dt, kind="ExternalInput")

    xsT_d = din("xsT", (H, TOKV))
    xs_d = din("xs", (TOKV, H))
    pT_d = din("pT", (H, EXTRA))
    wiT_d = din("wiT", (H, H))
    wl1T_d = din("wl1T", (H, H))
    wl2T_d = din("wl2T", (H, H))
    memWT_d = din("memWT", (H, H))
    wkeepT_d = din("wkeepT", (H, H))
    woT_d = din("woT", (H, H))
    wgxT_d = din("wgxT", (H, H))
    wgmT_d = din("wgmT", (H, H))
    w1T_d = din("w1T", (H, FF))
    w2T_d = din("w2T", (FF, H))
    bcols_d = {}
    for nm in ("bi", "bl1", "bl2", "memb", "bo", "bg", "b2",
               "g1", "be1", "g2", "be2", "g3", "be3"):
        bcols_d[nm] = din(nm + "_c", (P, HB), fp32)
    b1c_d = din("b1_c", (P, FFB), fp32)
    membrow_d = din("membrow", (1, H))
    cg_d = din("cg_c", (P, 1), fp32)
    mask0_d = din("mask0", (P, 256))
    maskr_d = din("maskr", (P, 256))
    ident_d = din("ident", (P, P))
    onescol_d = din("onescol", (P, 1))
    onesrow_d = din("onesrow", (1, P))
    zeros_d = din("zeros512", (P, 512))

    outT_d = nc.dram_tensor("outT", [H, TOKX], fp32, kind="ExternalOutput")

    with tile.TileContext(nc) as tc:
        with (
            nc.allow_low_precision(reason="float32r is bit-identical fp32"),
            tc.tile_pool(name="const", bufs=1) as const,
            tc.tile_pool(name="wpool", bufs=6) as wpool,
            tc.tile_pool(name="acts", bufs=1) as acts,
            tc.tile_pool(name="ps", bufs=1, space="PSUM") as ps,
            tc.tile_pool(name="scratch", bufs=1) as scratch,
            tc.tile_pool(name="dram", bufs=1, space="DRAM") as dram,
        ):
            # ---------- constants ----------
            ident_t = const.tile([P, P], F32R, tag="ident")
            nc.sync.dma_start(ident_t[:], ident_d[:, :].bitcast(F32R))
            ident = ident_t[:]
            ones_col_t = const.tile([P, 1], F32R, tag="ones_col")
            nc.sync.dma_start(ones_col_t[:], onescol_d[:, :].bitcast(F32R))
            ones_col = ones_col_t[:]
            ones_row_t = const.tile([1, P], F32R, tag="ones_row")
            nc.sync.dma_start(ones_row_t[:], onesrow_d[:, :].bitcast(F32R))
            ones_row = ones_row_t[:]
            bc = {}
            for nm in bcols_d:
                t = const.tile([P, HB], fp32, tag=f"bc_{nm}")
                nc.sync.dma_start(t[:], bcols_d[nm][:, :])
                bc[nm] = t
            b1c = const.tile([P, FFB], fp32, tag="b1c")
            nc.sync.dma_start(b1c[:], b1c_d[:, :])
            membrow = const.tile([1, H], F32R, tag="membrow")
            nc.sync.dma_start(membrow[:], membrow_d[:, :].bitcast(F32R))
            cg = const.tile([P, 1], fp32, tag="cg")
            nc.sync.dma_start(cg[:], cg_d[:, :])
            mask0 = const.tile([P, 256], F32R, tag="mask0")
            nc.sync.dma_start(mask0[:], mask0_d[:, :].bitcast(F32R))
            maskr = const.tile([P, 256], F32R, tag="maskr")
            nc.sync.dma_start(maskr[:], maskr_d[:, :].bitcast(F32R))
            pT = []
            for k in range(HB):
                t = const.tile([P, EXTRA], F32R, tag=f"pT{k}")
                nc.sync.dma_start(t[:], pT_d[k * P:(k + 1) * P, :].bitcast(F32R))
                pT.append(t)

            # ---------- helpers ----------
            name_ctr = [0]

            def uname(pfx):
                name_ctr[0] += 1
                return f"{pfx}_{name_ctr[0]}"

            def bank(tag, shape):
                return acts.tile(shape, F32R, tag=tag, name=uname(tag))

            def psum_mm(ci):
                return ps.tile([P, 512], fp32, tag=PSTAGS[ci],
                               bufs=(1 if ci == 2 else 2), name=uname("pmm"))

            def psum_row(tag, bufs):
                return ps.tile([1, 512], fp32, tag=tag, bufs=bufs,
                               name=uname(tag))

            def psum_tp():
                return ps.tile([P, P], F32R, tag="st1", bufs=2,
                               name=uname("tp"))

            def s512():
                return scratch.tile([P, 512], F32R, tag="s512", bufs=4,
                                    name=uname("s512"))

            def evict_copy(dst, src, bias=0.0, scale=1.0):
                func = AF.Copy if isinstance(bias, float) else AF.Identity
                nc.scalar.activation(dst, src, func, bias=bias, scale=scale)

            def evict_silu(dst, src, bias):
                if native_silu:
                    nc.scalar.activation(dst, src, AF.Silu, bias=bias)
                else:
                    t = s512()
                    w = dst.shape[-1]
                    nc.scalar.activation(t[:, :w], src, AF.Sigmoid, bias=bias)
                    nc.scalar.activation(dst, src, AF.Identity, bias=bias)
                    nc.vector.tensor_mul(dst, dst, t[:, :w])

            def mmT(w_dram, rhs_tiles, nk, chunks, evict, out_tags, ncols):
                """out[m] = evict(sum_k w_dram[kP:,mP:].T @ rhs_tiles[k])."""
                outs = []
                for m in range(len(out_tags)):
                    psums = [psum_mm(ci) for ci in range(len(chunks))]
                    for k in range(nk):
                        lt = wpool.tile([P, P], F32R, tag="lhsT",
                                        name=uname("lt"))
                        nc.sync.dma_start(
                            lt[:],
                            w_dram[k * P:(k + 1) * P,
                                   m * P:(m + 1) * P].bitcast(F32R))
                        for ci, (c0, cw) in enumerate(chunks):
                            nc.tensor.matmul(
                                psums[ci][:, :cw], lt[:],
                                rhs_tiles[k][:, c0:c0 + cw],
                                start=(k == 0), stop=(k == nk - 1))
                    t = bank(out_tags[m], [P, ncols])
                    for ci, (c0, cw) in enumerate(chunks):
                        evict(t[:, c0:c0 + cw], psums[ci][:, :cw], m)
                    outs.append(t)
                return outs

            def transpose_to_tokmajor(src_tiles, tags):
                """feature-major [P,TOKV] x HB -> token-major [P,H] x NTT."""
                outs = []
                for j in range(NTT):
                    t0 = j * P
                    tw = min(P, TOKV - t0)
                    t = bank(tags[j], [P, H])
                    for k in range(HB):
                        pt = psum_tp()
                        nc.tensor.transpose(
                            pt[:tw, :], src_tiles[k][:, t0:t0 + tw], ident)
                        evict_copy(t[:tw, k * P:(k + 1) * P], pt[:tw, :])
                    outs.append(t)
                return outs

            def layernorm(y_tiles, ncols, chunks, g_col, be_col, out_tiles,
                          borrow):
                """Feature-axis LN (feature-major layout); final y*g+be into
                out_tiles (may alias y_tiles). borrow = 5 bank tags."""
                mean = bank(borrow[0], [1, ncols])
                rs = bank(borrow[1], [1, ncols])
                for ci, (c0, cw) in enumerate(chunks):
                    s1p = psum_row("st1", 2)
                    s2p = psum_row("st2", 1)
                    for k in range(HB):
                        sq = s512()
                        nc.vector.tensor_mul(sq[:, :cw],
                                             y_tiles[k][:, c0:c0 + cw],
                                             y_tiles[k][:, c0:c0 + cw])
                        nc.tensor.matmul(s1p[:, :cw], ones_col,
                                         y_tiles[k][:, c0:c0 + cw],
                                         start=(k == 0), stop=(k == HB - 1))
                        nc.tensor.matmul(s2p[:, :cw], ones_col, sq[:, :cw],
                                         start=(k == 0), stop=(k == HB - 1))
                    nc.vector.tensor_scalar_mul(mean[:, c0:c0 + cw],
                                                s1p[:, :cw], 1.0 / H)
                    nc.vector.tensor_scalar_mul(rs[:, c0:c0 + cw],
                                                s2p[:, :cw], 1.0 / H)
                m2 = bank(borrow[2], [1, ncols])
                nc.vector.tensor_mul(m2[:], mean[:], mean[:])
                nc.vector.tensor_sub(rs[:], rs[:], m2[:])
                nc.vector.tensor_scalar_add(rs[:], rs[:], EPS_LN)
                nc.scalar.activation(rs[:], rs[:], AF.Sqrt)
                nc.vector.reciprocal(rs[:], rs[:])
                mean_b = bank(borrow[3], [P, ncols])
                rs_b = bank(borrow[4], [P, ncols])
                for src, dst in ((mean, mean_b), (rs, rs_b)):
                    for ci, (c0, cw) in enumerate(chunks):
                        pb = psum_mm(2)
                        nc.tensor.matmul(pb[:, :cw], ones_row,
                                         src[:, c0:c0 + cw],
                                         start=True, stop=True)
                        evict_copy(dst[:, c0:c0 + cw], pb[:, :cw])
                for k in range(HB):
                    nc.vector.tensor_sub(y_tiles[k][:], y_tiles[k][:],
                                         mean_b[:])
                    nc.vector.tensor_mul(y_tiles[k][:], y_tiles[k][:],
                                         rs_b[:])
                    nc.vector.tensor_scalar(
                        out_tiles[k][:], y_tiles[k][:],
                        g_col[:, k:k + 1], be_col[:, k:k + 1],
                        op0=A.mult, op1=A.add)
                return out_tiles

            # ---------- load xsT ----------
            xsT = []
            for k in range(HB):
                t = bank(f"X{k}", [P, TOKV])
                nc.sync.dma_start(t[:], xsT_d[k * P:(k + 1) * P, :].bitcast(F32R))
                xsT.append(t)

            # ---------- kn: row-normalized kv (feature-major) ----------
            rsn = bank("D1", [1, TOKV])
            for ci, (c0, cw) in enumerate(CH_V):
                ssp = psum_row("st1", 2)
                for k in range(HB):
                    sq = s512()
                    if c0 < TOKX:
                        nc.vector.tensor_mul(
                            sq[:, :cw],
                            xsT[k][:, HALO + c0:HALO + c0 + cw],
                            xsT[k][:, HALO + c0:HALO + c0 + cw])
                    else:
                        nc.vector.tensor_mul(sq[:, :cw], pT[k][:], pT[k][:])
                    nc.tensor.matmul(ssp[:, :cw], ones_col, sq[:, :cw],
                                     start=(k == 0), stop=(k == HB - 1))
                nc.scalar.activation(rsn[:, c0:c0 + cw], ssp[:, :cw], AF.Sqrt)
            nc.vector.tensor_scalar_max(rsn[:], rsn[:], EPS_NORM)
            nc.vector.reciprocal(rsn[:], rsn[:])
            rsn_b = bank("D0", [P, TOKV])
            for ci, (c0, cw) in enumerate(CH_V):
                pb = psum_mm(2)
                nc.tensor.matmul(pb[:, :cw], ones_row, rsn[:, c0:c0 + cw],
                                 start=True, stop=True)
                evict_copy(rsn_b[:, c0:c0 + cw], pb[:, :cw])
            knT = []
            for k in range(HB):
                t = bank(f"A{k}", [P, TOKV])
                nc.vector.tensor_mul(t[:, :TOKX],
                                     xsT[k][:, HALO:HALO + TOKX],
                                     rsn_b[:, :TOKX])
                nc.vector.tensor_mul(t[:, TOKX:], pT[k][:], rsn_b[:, TOKX:])
                knT.append(t)

            # ---------- memory chain ----------
            kpT = mmT(wiT_d, knT, HB, CH_V,
                      lambda d, s, m: evict_copy(
                          d, s, bias=bc["bi"][:, m:m + 1]),
                      [f"B{m}" for m in range(HB)], TOKV)

            kptok = transpose_to_tokmajor(kpT, [f"C{j}" for j in range(NTT)])

            h1T = mmT(wl1T_d, kpT, HB, CH_V,
                      lambda d, s, m: evict_silu(d, s, bc["bl1"][:, m:m + 1]),
                      [f"A{m}" for m in range(HB)], TOKV)

            h2T = mmT(wl2T_d, h1T, HB, CH_V,
                      lambda d, s, m: evict_silu(d, s, bc["bl2"][:, m:m + 1]),
                      [f"B{m}" for m in range(HB)], TOKV)

            h2tok = transpose_to_tokmajor(h2T, [f"D{j}" for j in range(NTT)])

            # diff (token-major, in place into kptok):
            # diff[t,m] = (pred_raw[t,m] + memb[m]) - (kp[t,m] + bi[m])
            for j in range(NTT):
                t0 = j * P
                tw = min(P, TOKV - t0)
                for ci, (c0, cw) in enumerate(CH_X):
                    pp = psum_mm(ci)
                    for k in range(HB):
                        rt = s512()
                        nc.sync.dma_start(
                            rt[:, :cw],
                            memWT_d[k * P:(k + 1) * P,
                                    c0:c0 + cw].bitcast(F32R))
                        nc.tensor.matmul(pp[:tw, :cw],
                                         h2T[k][:, t0:t0 + tw], rt[:, :cw],
                                         start=(k == 0), stop=False)
                    nc.tensor.matmul(pp[:tw, :cw], ones_row[:1, :tw],
                                     membrow[:, c0:c0 + cw],
                                     start=False, stop=True)
                    nc.vector.tensor_sub(kptok[j][:tw, c0:c0 + cw],
                                         pp[:tw, :cw],
                                         kptok[j][:tw, c0:c0 + cw])
            diff = kptok

            # grad: gradT[n,m] = cg * sum_t h2tok[t,n] * diff[t,m]
            gin = dram.tile([H, H], F32R)
            gout = dram.tile([H, H], F32R, addr_space="Shared")
            for a in range(HB):
                for ci, (c0, cw) in enumerate(CH_X):
                    pp = psum_mm(ci)
                    for j in range(NTT):
                        tw = min(P, TOKV - j * P)
                        nc.tensor.matmul(pp[:, :cw],
                                         h2tok[j][:tw, a * P:(a + 1) * P],
                                         diff[j][:tw, c0:c0 + cw],
                                         start=(j == 0), stop=(j == NTT - 1))
                    g = s512()
                    nc.scalar.activation(g[:, :cw], pp[:, :cw], AF.Copy,
                                         scale=cg[:, :1])
                    nc.sync.dma_start(gin[a * P:(a + 1) * P, c0:c0 + cw],
                                      g[:, :cw])

            nc.gpsimd.collective_compute(
                "AllReduce", A.add,
                replica_groups=[list(range(NCORES))],
                ins=[gin[:].opt()], outs=[gout[:].opt()],
            )

            # ---------- attention (overlaps the collective) ----------
            xstok = []
            for j in range(NTT):
                tw = min(P, TOKV - j * P)
                t = bank(f"C{j}", [P, H])
                nc.sync.dma_start(t[:tw, :], xs_d[j * P:j * P + tw, :].bitcast(F32R))
                xstok.append(t)

            y1 = [bank(f"D{k}", [P, TOKX]) for k in range(HB)]

            for qg in range(2):  # query groups of 512
                kts = []
                for kt in range(5):
                    t = bank(f"A{kt}", [P, 512])
                    nc.sync.dma_start(t[:], zeros_d[:, :].bitcast(F32R))
                    kts.append(t)
                for sj in range(4):  # 128-query subtiles
                    qt = qg * 4 + sj
                    kw = 144 if qt == 7 else 256  # key-window width
                    pp = psum_mm(0)
                    for k in range(HB):
                        nc.tensor.matmul(
                            pp[:, :kw],
                            xsT[k][:, HALO + qt * P:HALO + qt * P + P],
                            xsT[k][:, qt * P:qt * P + kw],
                            start=(k == 0), stop=(k == HB - 1))
                    probs = s512()
                    msk = mask0 if qt == 0 else maskr
                    nc.vector.tensor_add(probs[:, :kw], pp[:, :kw],
                                         msk[:, :kw])
                    mx = scratch.tile([P, 1], fp32, tag="mx", bufs=3,
                                      name=uname("mx"))
                    nc.vector.reduce_max(mx[:], probs[:, :kw],
                                         axis=mybir.AxisListType.X)
                    nc.vector.tensor_scalar_mul(mx[:], mx[:], -SCL)
                    nc.scalar.activation(probs[:, :kw], probs[:, :kw],
                                         AF.Exp, bias=mx[:, :1], scale=SCL)
                    sm = scratch.tile([P, 1], fp32, tag="sm", bufs=3,
                                      name=uname("sm"))
                    nc.vector.reduce_sum(sm[:], probs[:, :kw],
                                         axis=mybir.AxisListType.X)
                    nc.vector.reciprocal(sm[:], sm[:])
                    nc.vector.tensor_scalar_mul(probs[:, :kw], probs[:, :kw],
                                                sm[:, :1])
                    for half in range(2):
                        hw_ = min(P, kw - half * P)
                        pt = psum_tp()
                        nc.tensor.transpose(
                            pt[:hw_, :],
                            probs[:, half * P:half * P + hw_], ident)
                        evict_copy(
                            kts[sj + half][:hw_, sj * P:(sj + 1) * P],
                            pt[:hw_, :])
                for k in range(HB):
                    pp = psum_mm(1)
                    for kt in range(5):
                        ktw = min(P, TOKV - (qg * 4 + kt) * P)
                        nc.tensor.matmul(
                            pp[:],
                            xstok[qg * 4 + kt][:ktw, k * P:(k + 1) * P],
                            kts[kt][:ktw, :],
                            start=(kt == 0), stop=(kt == 4))
                    nc.vector.tensor_add(
                        y1[k][:, qg * 512:(qg + 1) * 512], pp[:],
                        xsT[k][:, HALO + qg * 512:HALO + (qg + 1) * 512])

            x1T = layernorm(y1, TOKX, CH_X, bc["g1"], bc["be1"], y1,
                            ["X0", "X1", "X2", "X3", "X4"])

            # ---------- W_new compose ----------
            wnew = []
            for k in range(HB):
                t = bank(f"C{k}", [P, H])
                nc.sync.dma_start(t[:], wkeepT_d[k * P:(k + 1) * P, :].bitcast(F32R))
                gr = bank("X5" if k % 2 == 0 else "X6", [P, H])
                nc.sync.dma_start(gr[:], gout[k * P:(k + 1) * P, :])
                nc.vector.tensor_add(t[:], t[:], gr[:])
                wnew.append(t)

            # ---------- memoT = W_new @ h2T + memb ----------
            memoT = []
            for m in range(HB):
                psums = [psum_mm(ci) for ci in range(len(CH_V))]
                for k in range(HB):
                    for ci, (c0, cw) in enumerate(CH_V):
                        nc.tensor.matmul(psums[ci][:, :cw],
                                         wnew[k][:, m * P:(m + 1) * P],
                                         h2T[k][:, c0:c0 + cw],
                                         start=(k == 0), stop=(k == HB - 1))
                t = bank(f"A{m}", [P, TOKV])
                for ci, (c0, cw) in enumerate(CH_V):
                    evict_copy(t[:, c0:c0 + cw], psums[ci][:, :cw],
                               bias=bc["memb"][:, m:m + 1])
                memoT.append(t)

            # ---------- moT = Wo @ memoT + bo ----------
            moT = mmT(woT_d, memoT, HB, CH_V,
                      lambda d, s, m: evict_copy(
                          d, s, bias=bc["bo"][:, m:m + 1]),
                      [f"B{m}" for m in range(HB)], TOKV)

            # ---------- gate = sigmoid(Wgx@x1 + Wgm@mo + bg) ----------
            gateT = []
            for m in range(HB):
                psums = [psum_mm(ci) for ci in range(len(CH_X))]
                for half, (wd, rhs) in enumerate(
                        ((wgxT_d, x1T), (wgmT_d, moT))):
                    for k in range(HB):
                        lt = wpool.tile([P, P], F32R, tag="lhsT",
                                        name=uname("lt"))
                        nc.sync.dma_start(
                            lt[:], wd[k * P:(k + 1) * P,
                                      m * P:(m + 1) * P].bitcast(F32R))
                        for ci, (c0, cw) in enumerate(CH_X):
                            nc.tensor.matmul(
                                psums[ci][:, :cw], lt[:],
                                rhs[k][:, c0:c0 + cw],
                                start=(half == 0 and k == 0),
                                stop=(half == 1 and k == HB - 1))
                t = bank(f"X{m}", [P, TOKX])
                for ci, (c0, cw) in enumerate(CH_X):
                    nc.scalar.activation(t[:, c0:c0 + cw], psums[ci][:, :cw],
                                         AF.Sigmoid,
                                         bias=bc["bg"][:, m:m + 1])
                gateT.append(t)

            # ---------- combine + LN2 -> x2T ----------
            y2 = []
            for k in range(HB):
                d = bank("A6" if k % 2 == 0 else "A7", [P, TOKX])
                nc.vector.tensor_sub(d[:], x1T[k][:], moT[k][:, :TOKX])
                nc.vector.tensor_mul(d[:], d[:], gateT[k][:])
                t = bank(f"C{k}", [P, TOKX])
                nc.vector.tensor_add(t[:], x1T[k][:], moT[k][:, :TOKX])
                nc.vector.tensor_add(t[:], t[:], d[:])
                y2.append(t)
            x2T = layernorm(y2, TOKX, CH_X, bc["g2"], bc["be2"], y2,
                            ["A0", "A1", "A2", "A3", "A4"])

            # ---------- FFN ----------
            ffs_tags = ([f"B{j}" for j in range(8)]
                        + [f"D{j}" for j in range(8)]
                        + [f"X{k}" for k in range(8)]
                        + [f"A{k}" for k in range(8)])
            ffsT = mmT(w1T_d, x2T, HB, CH_X,
                       lambda d, s, m: evict_silu(d, s, b1c[:, m:m + 1]),
                       ffs_tags, TOKX)

            # ff2 + residual accumulated into x2T
            for m in range(HB):
                psums = [psum_mm(ci) for ci in range(len(CH_X))]
                for k in range(FFB):
                    lt = wpool.tile([P, P], F32R, tag="lhsT",
                                    name=uname("lt"))
                    nc.sync.dma_start(
                        lt[:], w2T_d[k * P:(k + 1) * P,
                                     m * P:(m + 1) * P].bitcast(F32R))
                    for ci, (c0, cw) in enumerate(CH_X):
                        nc.tensor.matmul(psums[ci][:, :cw], lt[:],
                                         ffsT[k][:, c0:c0 + cw],
                                         start=(k == 0), stop=(k == FFB - 1))
                for ci, (c0, cw) in enumerate(CH_X):
                    ft = s512()
                    nc.scalar.activation(ft[:, :cw], psums[ci][:, :cw],
                                         AF.Identity,
                                         bias=bc["b2"][:, m:m + 1])
                    nc.vector.tensor_add(x2T[m][:, c0:c0 + cw],
                                         x2T[m][:, c0:c0 + cw], ft[:, :cw])

            # ---------- LN3 -> outT ----------
            outt = [bank(f"D{k % 3}", [P, TOKX]) for k in range(HB)]
            out3 = layernorm(x2T, TOKX, CH_X, bc["g3"], bc["be3"], outt,
                             ["B0", "B1", "B2", "B3", "B4"])
            for k in range(HB):
                nc.sync.dma_start(outT_d[k * P:(k + 1) * P, :].bitcast(F32R),
                                  out3[k][:])

    nc.compile()
    return nc


_NC_CACHE = {}


def _get_nc(native_silu: bool):
    key = bool(native_silu)
    if key not in _NC_CACHE:
        _NC_CACHE[key] = _build(native_silu)
    return _NC_CACHE[key]


def _host_prep(inputs):
    """Build the 8 per-core input maps from the full problem inputs."""
    f = np.float32
    x = np.asarray(inputs["x"], f)
    Pm = np.asarray(inputs["P"], f)
    fgate = float(np.asarray(inputs["fgate"]).reshape(-1)[0])
    lrate = float(np.asarray(inputs["lrate"]).reshape(-1)[0])

    def T(a):
        return np.ascontiguousarray(np.asarray(a, f).T)

    shared = {
        "wiT": T(inputs["Wi"]), "wl1T": T(inputs["Wl1"]),
        "wl2T": T(inputs["Wl2"]), "memWT": T(inputs["mem_W"]),
        "woT": T(inputs["Wo"]),
        "wgxT": T(np.asarray(inputs["Wg"], f)[:, :H]),
        "wgmT": T(np.asarray(inputs["Wg"], f)[:, H:]),
        "w1T": T(inputs["W1"]), "w2T": T(inputs["W2"]),
        "wkeepT": np.ascontiguousarray(
            (1.0 - fgate) * np.asarray(inputs["mem_W"], f).T),
        "membrow": np.asarray(inputs["mem_b"], f).reshape(1, H).copy(),
        "cg_c": np.full((P, 1), 2.0 * lrate / (NKV * H), f),
        "b1_c": np.ascontiguousarray(
            np.asarray(inputs["b1"], f).reshape(FFB, P).T),
    }
    for nm, key in (("bi", "bi"), ("bl1", "bl1"), ("bl2", "bl2"),
                    ("memb", "mem_b"), ("bo", "bo"), ("bg", "bg"),
                    ("b2", "b2"), ("g1", "g1"), ("be1", "be1"),
                    ("g2", "g2"), ("be2", "be2"), ("g3", "g3"),
                    ("be3", "be3")):
        shared[nm + "_c"] = np.ascontiguousarray(
            np.asarray(inputs[key], f).reshape(HB, P).T)

    r = np.arange(P)[:, None]
    c = np.arange(256)[None, :]
    band = (c >= r) & (c <= r + WIN)
    shared["maskr"] = np.where(band, 0.0, NEG).astype(f)
    shared["mask0"] = np.where(band & (c >= HALO), 0.0, NEG).astype(f)
    shared["ident"] = np.eye(P, dtype=f)
    shared["onescol"] = np.ones((P, 1), f)
    shared["onesrow"] = np.ones((1, P), f)
    shared["zeros512"] = np.zeros((P, 512), f)

    in_maps = []
    for core in range(NCORES):
        b, blk = divmod(core, 4)
        t0 = blk * TOKX
        xs = np.zeros((TOKV, H), f)
        xs[HALO:HALO + TOKX] = x[b, t0:t0 + TOKX]
        if blk > 0:
            xs[:HALO] = x[b, t0 - HALO:t0]
        pT = np.zeros((H, EXTRA), f)
        if blk == 3:
            pT[:, :SLOTS] = Pm.T
        m = dict(shared)
        m["xs"] = xs
        m["xsT"] = np.ascontiguousarray(xs.T)
        m["pT"] = pT
        in_maps.append(m)
    return in_maps


def _assemble(results):
    out = np.empty((B, S, H), np.float32)
    for core in range(NCORES):
        b, blk = divmod(core, 4)
        out[b, blk * TOKX:(blk + 1) * TOKX, :] = \
            np.asarray(results[core]["outT"]).T
    return out


def kernel(**inputs) -> np.ndarray:
    from concourse.bass_utils import run_bass_kernel_spmd
    native = os.environ.get("MEMBLK_NATIVE_SILU", "1") == "1"
    nc = _get_nc(native)
    in_maps = _host_prep(inputs)
    res = run_bass_kernel_spmd(nc, in_maps, list(range(NCORES)))
    return _assemble(res.results)


def kernel_sim(**inputs) -> np.ndarray:
    """CoreSim path for correctness validation (no hardware)."""
    from concourse.bass_interp import MultiCoreSim
    nc = _get_nc(False)
    in_maps = _host_prep(inputs)
    sim = MultiCoreSim(nc, NCORES)
    for i in range(NCORES):
        for k, v in in_maps[i].items():
            sim.cores[i].tensor(k)[:] = v
    sim.simulate(check_with_hw=False)
    results = [{"outT": np.array(sim.cores[i].tensor("outT"))}
               for i in range(NCORES)]
    return _assemble(results)
